# revision 8
# baseline (speedup 1.0000x reference)
"""ArcFace head (B=1024, D=512, C=100000) on 8 TRN2 NeuronCores.

Sharding: tensor-parallel along the num_classes axis (partial-FC ArcFace).
Each core holds a [D, C/8] slice of the (pre-normalized, pre-transposed)
weight and computes its [B, C/8] slice of S * cosine via a bf16 TensorE
matmul with fp32 PSUM accumulation. Embeddings (normalized, scaled by S,
transposed to [D, B]) are broadcast to all cores. The per-row additive
angular margin touches exactly B elements of the [B, C] output, so it is
applied on the host after the gather (exact trig identity:
cos(theta + m) = c*cos(m) - sqrt(1-c^2)*sin(m)).
"""

import os

import numpy as np
import ml_dtypes

import concourse.bass as bass
import concourse.mybir as mybir
from concourse import bacc
from concourse.tile import TileContext
from concourse.bass import ts
from concourse.bass_utils import run_bass_kernel_spmd

# Problem constants (hardcoded per spec)
B, D, C = 1024, 512, 100000
NCORES = 8
CS = C // NCORES          # 12500 classes per core
S, MARGIN, EPS = 30.0, 0.5, 1e-7

P = 128                   # partitions
KS = D // P               # 4 k-subtiles
MS = B // P               # 8 m-subtiles
NT = 512                  # n tile (one PSUM bank of fp32)
N_TILES = (CS + NT - 1) // NT  # 25 (last tile 212 wide)

BF16 = mybir.dt.bfloat16
_bf16_np = ml_dtypes.bfloat16


def build_nc():
    nc = bacc.Bacc(None, target_bir_lowering=False)
    embT = nc.dram_tensor("embT", [D, B], BF16, kind="ExternalInput")
    wT = nc.dram_tensor("wT", [D, CS], BF16, kind="ExternalInput")
    out = nc.dram_tensor("out", [B, CS], BF16, kind="ExternalOutput")

    WARMUP_MMS = 40

    with TileContext(nc) as tc:
        with (
            tc.tile_pool(name="emb", bufs=1) as epool,
            tc.tile_pool(name="w", bufs=3) as wpool,
            tc.tile_pool(name="o", bufs=6) as opool,
            tc.tile_pool(name="ps", bufs=8, space="PSUM") as pspool,
        ):
            embT_r = embT[:].rearrange("(ko p) b -> p ko b", p=P)
            wT_r = wT[:].rearrange("(ko p) c -> p ko c", p=P)
            out_r = out[:].rearrange("(mo p) c -> p mo c", p=P)

            # PE warm-up: dummy matmuls run during the initial DMA wait so the
            # HAM clock gate is at 2.4 GHz when the real MM stream starts.
            dummy = epool.tile([P, 64], BF16, tag="dummy")
            nc.vector.memset(dummy[:], 0.0)
            wps = pspool.tile([P, NT], mybir.dt.float32, tag="ps")
            for _ in range(WARMUP_MMS):
                nc.tensor.matmul(
                    wps[:64, :64], lhsT=dummy[:64, :], rhs=dummy[:64, :],
                    start=True, stop=True,
                )

            # per-k embedding tiles so the first matmul waits on 256KB, not 1MB
            emb_sb = []
            for k in range(KS):
                e = epool.tile([P, B], BF16, tag=f"emb{k}")
                nc.sync.dma_start(out=e[:], in_=embT_r[:, k, :])
                emb_sb.append(e)

            # ragged 212-wide tile first so the tail is a full-efficiency tile
            order = [N_TILES - 1] + list(range(N_TILES - 1))
            for idx, n in enumerate(order):
                n0 = n * NT
                nw = min(NT, CS - n0)
                w_sb = wpool.tile([P, KS, NT], BF16, tag="w")
                if idx < 2:
                    # chunk the first tiles per-k: subtile deps let the k=0
                    # matmuls start before the rest of the tile lands
                    for k in range(KS):
                        nc.sync.dma_start(
                            out=w_sb[:, k, :nw], in_=wT_r[:, k, n0 : n0 + nw]
                        )
                else:
                    nc.sync.dma_start(
                        out=w_sb[:, :, :nw], in_=wT_r[:, :, n0 : n0 + nw]
                    )
                o_sb = opool.tile([P, MS, NT], BF16, tag="o")
                for m in range(MS):
                    ps = pspool.tile([P, NT], mybir.dt.float32, tag="ps", name=f"ps_{n}_{m}")
                    for k in range(KS):
                        nc.tensor.matmul(
                            ps[:, :nw],
                            lhsT=emb_sb[k][:, ts(m, P)],
                            rhs=w_sb[:, k, :nw],
                            start=(k == 0),
                            stop=(k == KS - 1),
                        )
                    # split PSUM->SBUF cast copies between ACT and DVE
                    if m % 2 == 0:
                        nc.scalar.copy(out=o_sb[:, m, :nw], in_=ps[:, :nw])
                    else:
                        nc.vector.tensor_copy(out=o_sb[:, m, :nw], in_=ps[:, :nw])
                    # half-tile output DMAs: second half can start while the
                    # next tile's matmuls run; keeps the kernel tail short
                    if m == MS // 2 - 1:
                        nc.sync.dma_start(
                            out=out_r[:, 0 : MS // 2, n0 : n0 + nw],
                            in_=o_sb[:, 0 : MS // 2, :nw],
                        )
                    elif m == MS - 1:
                        nc.sync.dma_start(
                            out=out_r[:, MS // 2 : MS, n0 : n0 + nw],
                            in_=o_sb[:, MS // 2 : MS, :nw],
                        )
    nc.finalize()
    return nc


_NC_CACHE = []


def _get_nc():
    if not _NC_CACHE:
        _NC_CACHE.append(build_nc())
    return _NC_CACHE[0]


def _prep_in_maps(embeddings, weight):
    # normalize on host (fp32), fold the ArcFace scale S into the embeddings
    en = embeddings / np.maximum(
        np.linalg.norm(embeddings, axis=1, keepdims=True), 1e-12
    )
    wn = weight / np.maximum(np.linalg.norm(weight, axis=1, keepdims=True), 1e-12)
    embT = np.ascontiguousarray((S * en).T).astype(_bf16_np)  # [D, B]
    wTn = wn.T  # [D, C] view
    in_maps = []
    for i in range(NCORES):
        shard = np.ascontiguousarray(wTn[:, i * CS : (i + 1) * CS]).astype(_bf16_np)
        in_maps.append({"embT": embT, "wT": shard})
    return in_maps


def run_device(embeddings, weight, **spmd_kwargs):
    """Runs the device part; returns (full S*cosine [B, C] fp32, raw results)."""
    if not spmd_kwargs.get("trace"):
        # the axon NTFF-profile hook may be absent in this image; make sure an
        # ambient BASS_TRACE env var can't route us onto that path
        os.environ.setdefault("BASS_NEVER_TRACE", "1")
    nc = _get_nc()
    in_maps = _prep_in_maps(embeddings, weight)
    res = run_bass_kernel_spmd(nc, in_maps, core_ids=list(range(NCORES)), **spmd_kwargs)
    out = np.concatenate(
        [np.asarray(res.results[i]["out"]).astype(np.float32) for i in range(NCORES)],
        axis=1,
    )
    return out, res


def apply_margin(out, labels):
    rows = np.arange(B)
    lab = np.asarray(labels).astype(np.int64)
    c = np.clip(out[rows, lab] / S, -1.0 + EPS, 1.0 - EPS)
    out[rows, lab] = S * (c * np.cos(MARGIN) - np.sqrt(1.0 - c * c) * np.sin(MARGIN))
    return out


def kernel(embeddings, weight, labels):
    embeddings = np.asarray(embeddings, dtype=np.float32)
    weight = np.asarray(weight, dtype=np.float32)
    out, _ = run_device(embeddings, weight)
    return apply_margin(out, labels)


# revision 10
# speedup vs baseline: 1.1969x; 1.1969x over previous
"""ArcFace head (B=1024, D=512, C=100000) on 8 TRN2 NeuronCores.

Sharding: tensor-parallel along the num_classes axis (partial-FC ArcFace).
Each core holds a [D, C/8] slice of the (pre-normalized, pre-transposed)
weight and computes its [B, C/8] slice of S * cosine via a bf16 TensorE
matmul with fp32 PSUM accumulation. Embeddings (normalized, scaled by S,
transposed to [D, B]) are broadcast to all cores. The per-row additive
angular margin touches exactly B elements of the [B, C] output, so it is
applied on the host after the gather (exact trig identity:
cos(theta + m) = c*cos(m) - sqrt(1-c^2)*sin(m)).
"""

import os

import numpy as np
import ml_dtypes

import concourse.bass as bass
import concourse.mybir as mybir
from concourse import bacc
from concourse.tile import TileContext
from concourse.bass import ts
from concourse.bass_utils import run_bass_kernel_spmd

# Problem constants (hardcoded per spec)
B, D, C = 1024, 512, 100000
NCORES = 8
CS = C // NCORES          # 12500 classes per core
S, MARGIN, EPS = 30.0, 0.5, 1e-7

P = 128                   # partitions
KS = D // P               # 4 k-subtiles
MS = B // P               # 8 m-subtiles
NT = 512                  # n tile (one PSUM bank of fp32)
N_TILES = (CS + NT - 1) // NT  # 25 (last tile 212 wide)

BF16 = mybir.dt.bfloat16
_bf16_np = ml_dtypes.bfloat16


def build_nc():
    nc = bacc.Bacc(None, target_bir_lowering=False)
    embT = nc.dram_tensor("embT", [D, B], BF16, kind="ExternalInput")
    wT = nc.dram_tensor("wT", [D, CS], BF16, kind="ExternalInput")
    out = nc.dram_tensor("out", [B, CS], BF16, kind="ExternalOutput")

    WARMUP_MMS = 40

    with TileContext(nc) as tc:
        with (
            tc.tile_pool(name="emb", bufs=1) as epool,
            tc.tile_pool(name="w", bufs=3) as wpool,
            tc.tile_pool(name="o", bufs=3) as opool,
            tc.tile_pool(name="ps", bufs=8, space="PSUM") as pspool,
        ):
            embT_r = embT[:].rearrange("(ko p) b -> p ko b", p=P)
            wT_r = wT[:].rearrange("(ko p) c -> p ko c", p=P)
            out_r = out[:].rearrange("(mo p) c -> p mo c", p=P)

            # PE warm-up: dummy matmuls run during the initial DMA wait so the
            # HAM clock gate is at 2.4 GHz when the real MM stream starts.
            dummy = epool.tile([P, 64], BF16, tag="dummy")
            nc.vector.memset(dummy[:], 0.0)
            wps = pspool.tile([P, NT], mybir.dt.float32, tag="ps")
            for _ in range(WARMUP_MMS):
                nc.tensor.matmul(
                    wps[:64, :64], lhsT=dummy[:64, :], rhs=dummy[:64, :],
                    start=True, stop=True,
                )

            # per-k embedding tiles so the first matmul waits on 256KB, not 1MB
            emb_sb = []
            for k in range(KS):
                e = epool.tile([P, B], BF16, tag=f"emb{k}")
                nc.sync.dma_start(out=e[:], in_=embT_r[:, k, :])
                emb_sb.append(e)

            # super-tiles of 1024 columns -> 2KB DMA descriptors (vs 1KB at
            # 512): roughly halves DMA engine occupancy for the same bytes.
            # The ragged 212-wide tile goes first so the tail is efficient.
            supers = [(12 * 2 * NT, CS - 12 * 2 * NT)] + [
                (i * 2 * NT, 2 * NT) for i in range(12)
            ]
            first = True
            for n0, nw in supers:
                w_sb = wpool.tile([P, KS, 2 * NT], BF16, tag="w")
                if first:
                    # chunk the first tile per-k: subtile deps let the k=0
                    # matmuls start before the rest of the tile lands
                    for k in range(KS):
                        nc.sync.dma_start(
                            out=w_sb[:, k, :nw], in_=wT_r[:, k, n0 : n0 + nw]
                        )
                    first = False
                else:
                    nc.sync.dma_start(
                        out=w_sb[:, :, :nw], in_=wT_r[:, :, n0 : n0 + nw]
                    )
                o_sb = opool.tile([P, MS, 2 * NT], BF16, tag="o")
                for h in range(2):
                    h0 = h * NT
                    hw = min(NT, nw - h0)
                    if hw <= 0:
                        continue
                    for m in range(MS):
                        ps = pspool.tile(
                            [P, NT], mybir.dt.float32, tag="ps", name=f"ps_{n0}_{h}_{m}"
                        )
                        for k in range(KS):
                            nc.tensor.matmul(
                                ps[:, :hw],
                                lhsT=emb_sb[k][:, ts(m, P)],
                                rhs=w_sb[:, k, h0 : h0 + hw],
                                start=(k == 0),
                                stop=(k == KS - 1),
                            )
                        # split PSUM->SBUF cast copies between ACT and DVE
                        if m % 2 == 0:
                            nc.scalar.copy(
                                out=o_sb[:, m, h0 : h0 + hw], in_=ps[:, :hw]
                            )
                        else:
                            nc.vector.tensor_copy(
                                out=o_sb[:, m, h0 : h0 + hw], in_=ps[:, :hw]
                            )
                        # half-tile output DMAs (by m-range, keeping rows
                        # contiguous): second half streams out while the next
                        # tile computes; keeps the kernel tail short
                        last_h = (h == 1) or (nw <= NT)
                        if last_h and m == MS // 2 - 1:
                            nc.sync.dma_start(
                                out=out_r[:, 0 : MS // 2, n0 : n0 + nw],
                                in_=o_sb[:, 0 : MS // 2, :nw],
                            )
                        elif last_h and m == MS - 1:
                            nc.sync.dma_start(
                                out=out_r[:, MS // 2 : MS, n0 : n0 + nw],
                                in_=o_sb[:, MS // 2 : MS, :nw],
                            )
    nc.finalize()
    return nc


_NC_CACHE = []


def _get_nc():
    if not _NC_CACHE:
        _NC_CACHE.append(build_nc())
    return _NC_CACHE[0]


def _prep_in_maps(embeddings, weight):
    # normalize on host (fp32), fold the ArcFace scale S into the embeddings
    en = embeddings / np.maximum(
        np.linalg.norm(embeddings, axis=1, keepdims=True), 1e-12
    )
    wn = weight / np.maximum(np.linalg.norm(weight, axis=1, keepdims=True), 1e-12)
    embT = np.ascontiguousarray((S * en).T).astype(_bf16_np)  # [D, B]
    wTn = wn.T  # [D, C] view
    in_maps = []
    for i in range(NCORES):
        shard = np.ascontiguousarray(wTn[:, i * CS : (i + 1) * CS]).astype(_bf16_np)
        in_maps.append({"embT": embT, "wT": shard})
    return in_maps


def run_device(embeddings, weight, **spmd_kwargs):
    """Runs the device part; returns (full S*cosine [B, C] fp32, raw results)."""
    if not spmd_kwargs.get("trace"):
        # the axon NTFF-profile hook may be absent in this image; make sure an
        # ambient BASS_TRACE env var can't route us onto that path
        os.environ.setdefault("BASS_NEVER_TRACE", "1")
    nc = _get_nc()
    in_maps = _prep_in_maps(embeddings, weight)
    res = run_bass_kernel_spmd(nc, in_maps, core_ids=list(range(NCORES)), **spmd_kwargs)
    out = np.concatenate(
        [np.asarray(res.results[i]["out"]).astype(np.float32) for i in range(NCORES)],
        axis=1,
    )
    return out, res


def apply_margin(out, labels):
    rows = np.arange(B)
    lab = np.asarray(labels).astype(np.int64)
    c = np.clip(out[rows, lab] / S, -1.0 + EPS, 1.0 - EPS)
    out[rows, lab] = S * (c * np.cos(MARGIN) - np.sqrt(1.0 - c * c) * np.sin(MARGIN))
    return out


def kernel(embeddings, weight, labels):
    embeddings = np.asarray(embeddings, dtype=np.float32)
    weight = np.asarray(weight, dtype=np.float32)
    out, _ = run_device(embeddings, weight)
    return apply_margin(out, labels)


# revision 14
# speedup vs baseline: 1.2205x; 1.0197x over previous
"""ArcFace head (B=1024, D=512, C=100000) on 8 TRN2 NeuronCores.

Sharding: tensor-parallel along the num_classes axis (partial-FC ArcFace).
Each core holds a [D, C/8] slice of the (pre-normalized, pre-transposed)
weight and computes its [B, C/8] slice of S * cosine via a bf16 TensorE
matmul with fp32 PSUM accumulation. Embeddings (normalized, scaled by S,
transposed to [D, B]) are broadcast to all cores. The per-row additive
angular margin touches exactly B elements of the [B, C] output, so it is
applied on the host after the gather (exact trig identity:
cos(theta + m) = c*cos(m) - sqrt(1-c^2)*sin(m)).
"""

import os

import numpy as np
import ml_dtypes

import concourse.bass as bass
import concourse.mybir as mybir
from concourse import bacc
from concourse.tile import TileContext
from concourse.bass import ts
from concourse.bass_utils import run_bass_kernel_spmd

# Problem constants (hardcoded per spec)
B, D, C = 1024, 512, 100000
NCORES = 8
CS = C // NCORES          # 12500 classes per core
S, MARGIN, EPS = 30.0, 0.5, 1e-7

P = 128                   # partitions
KS = D // P               # 4 k-subtiles
MS = B // P               # 8 m-subtiles
NT = 512                  # n tile (one PSUM bank of fp32)
N_TILES = (CS + NT - 1) // NT  # 25 (last tile 212 wide)

BF16 = mybir.dt.bfloat16
_bf16_np = ml_dtypes.bfloat16


def build_nc():
    nc = bacc.Bacc(None, target_bir_lowering=False)
    embT = nc.dram_tensor("embT", [D, B], BF16, kind="ExternalInput")
    wT = nc.dram_tensor("wT", [D, CS], BF16, kind="ExternalInput")
    out = nc.dram_tensor("out", [B, CS], BF16, kind="ExternalOutput")

    WARMUP_MMS = 60

    with TileContext(nc) as tc:
        with (
            tc.tile_pool(name="emb", bufs=1) as epool,
            tc.tile_pool(name="w", bufs=3) as wpool,
            tc.tile_pool(name="o", bufs=3) as opool,
            tc.tile_pool(name="ps", bufs=8, space="PSUM") as pspool,
        ):
            embT_r = embT[:].rearrange("(ko p) b -> p ko b", p=P)
            wT_r = wT[:].rearrange("(ko p) c -> p ko c", p=P)
            out_r = out[:].rearrange("(mo p) c -> p mo c", p=P)

            # PE warm-up: dummy matmuls run during the initial DMA wait so the
            # HAM clock gate is at 2.4 GHz when the real MM stream starts.
            dummy = epool.tile([P, 64], BF16, tag="dummy")
            nc.vector.memset(dummy[:], 0.0)
            wps = pspool.tile([P, NT], mybir.dt.float32, tag="ps")
            for _ in range(WARMUP_MMS):
                nc.tensor.matmul(
                    wps[:64, :64], lhsT=dummy[:64, :], rhs=dummy[:64, :],
                    start=True, stop=True,
                )

            # per-k embedding tiles so the first matmul waits on 256KB, not 1MB
            emb_sb = []
            for k in range(KS):
                e = epool.tile([P, B], BF16, tag=f"emb{k}")
                nc.sync.dma_start(out=e[:], in_=embT_r[:, k, :])
                emb_sb.append(e)

            # super-tiles of 1024 columns -> 2KB DMA descriptors (vs 1KB at
            # 512): roughly halves DMA engine occupancy for the same bytes.
            # Order: one full tile first (chunked per-k so matmuls start on
            # partial data), the ragged 212-wide tile second (its inefficient
            # 424B-descriptor DMAs hide mid-stream), two narrow 512 tiles
            # last so the kernel tail flushes a small final transfer.
            supers = (
                [(0, 2 * NT)]
                + [(12 * 2 * NT, CS - 12 * 2 * NT)]
                + [(i * 2 * NT, 2 * NT) for i in range(1, 11)]
                + [(11 * 2 * NT, NT), (11 * 2 * NT + NT, NT)]
            )
            first = True
            for n0, nw in supers:
                w_sb = wpool.tile([P, KS, 2 * NT], BF16, tag="w")
                if first:
                    # chunk the first tile per-k: subtile deps let the k=0
                    # matmuls start before the rest of the tile lands
                    for k in range(KS):
                        nc.sync.dma_start(
                            out=w_sb[:, k, :nw], in_=wT_r[:, k, n0 : n0 + nw]
                        )
                    first = False
                else:
                    nc.sync.dma_start(
                        out=w_sb[:, :, :nw], in_=wT_r[:, :, n0 : n0 + nw]
                    )
                o_sb = opool.tile([P, MS, 2 * NT], BF16, tag="o")
                for h in range(2):
                    h0 = h * NT
                    hw = min(NT, nw - h0)
                    if hw <= 0:
                        continue
                    for m in range(MS):
                        ps = pspool.tile(
                            [P, NT], mybir.dt.float32, tag="ps", name=f"ps_{n0}_{h}_{m}"
                        )
                        for k in range(KS):
                            nc.tensor.matmul(
                                ps[:, :hw],
                                lhsT=emb_sb[k][:, ts(m, P)],
                                rhs=w_sb[:, k, h0 : h0 + hw],
                                start=(k == 0),
                                stop=(k == KS - 1),
                            )
                        # split PSUM->SBUF cast copies between ACT and DVE
                        if m % 2 == 0:
                            nc.scalar.copy(
                                out=o_sb[:, m, h0 : h0 + hw], in_=ps[:, :hw]
                            )
                        else:
                            nc.vector.tensor_copy(
                                out=o_sb[:, m, h0 : h0 + hw], in_=ps[:, :hw]
                            )
                        # half-tile output DMAs (by m-range, keeping rows
                        # contiguous): second half streams out while the next
                        # tile computes; keeps the kernel tail short
                        last_h = (h == 1) or (nw <= NT)
                        if last_h and m == MS // 2 - 1:
                            nc.sync.dma_start(
                                out=out_r[:, 0 : MS // 2, n0 : n0 + nw],
                                in_=o_sb[:, 0 : MS // 2, :nw],
                            )
                        elif last_h and m == MS - 1:
                            nc.sync.dma_start(
                                out=out_r[:, MS // 2 : MS, n0 : n0 + nw],
                                in_=o_sb[:, MS // 2 : MS, :nw],
                            )
    nc.finalize()
    return nc


_NC_CACHE = []


def _get_nc():
    if not _NC_CACHE:
        _NC_CACHE.append(build_nc())
    return _NC_CACHE[0]


def _prep_in_maps(embeddings, weight):
    # normalize on host (fp32), fold the ArcFace scale S into the embeddings
    en = embeddings / np.maximum(
        np.linalg.norm(embeddings, axis=1, keepdims=True), 1e-12
    )
    wn = weight / np.maximum(np.linalg.norm(weight, axis=1, keepdims=True), 1e-12)
    embT = np.ascontiguousarray((S * en).T).astype(_bf16_np)  # [D, B]
    wTn = wn.T  # [D, C] view
    in_maps = []
    for i in range(NCORES):
        shard = np.ascontiguousarray(wTn[:, i * CS : (i + 1) * CS]).astype(_bf16_np)
        in_maps.append({"embT": embT, "wT": shard})
    return in_maps


def run_device(embeddings, weight, **spmd_kwargs):
    """Runs the device part; returns (full S*cosine [B, C] fp32, raw results)."""
    if not spmd_kwargs.get("trace"):
        # the axon NTFF-profile hook may be absent in this image; make sure an
        # ambient BASS_TRACE env var can't route us onto that path
        os.environ.setdefault("BASS_NEVER_TRACE", "1")
    nc = _get_nc()
    in_maps = _prep_in_maps(embeddings, weight)
    res = run_bass_kernel_spmd(nc, in_maps, core_ids=list(range(NCORES)), **spmd_kwargs)
    out = np.concatenate(
        [np.asarray(res.results[i]["out"]).astype(np.float32) for i in range(NCORES)],
        axis=1,
    )
    return out, res


def apply_margin(out, labels):
    rows = np.arange(B)
    lab = np.asarray(labels).astype(np.int64)
    c = np.clip(out[rows, lab] / S, -1.0 + EPS, 1.0 - EPS)
    out[rows, lab] = S * (c * np.cos(MARGIN) - np.sqrt(1.0 - c * c) * np.sin(MARGIN))
    return out


def kernel(embeddings, weight, labels):
    embeddings = np.asarray(embeddings, dtype=np.float32)
    weight = np.asarray(weight, dtype=np.float32)
    out, _ = run_device(embeddings, weight)
    return apply_margin(out, labels)
